# revision 1
# baseline (speedup 1.0000x reference)
"""Bass/Tile kernel for causal self-attention with Shaw-style relative position
embeddings (Transformer-XL skew), data-parallel over batch on 8 cores.

Per-core computation (batch b):
  qkv = x @ Wattn + battn ; q,k,v per head (H heads, HD=64)
  att = softmax(causal(q k^T/sqrt(hd) + skew(q embk^T)))
  y   = (att @ v + skew2(att) @ embv) @ Wproj + bproj

Key realization of the two skews: per-partition-shifted ("diagonal") access
patterns are not expressible for SBUF on real DMA hardware, but they are for
flat DRAM. So qE = q@embk^T and P = exp(logits) round-trip through small DRAM
scratch buffers, and the skewed operands are fetched with diagonal DRAM reads
(row-contiguous, positive unit inner stride).

Coordinates: attention columns are stored time-REVERSED (c = T-1-s) so both
diagonal reads are contiguous forward:
  att2r[t, c] = qE[t, t-T+1+c]   (forward qE buffer; front pad = -1e30 also
                                  implements the causal mask via exp -> 0)
  PS[t, r]    = Pr[t, T-1-t+r]   (reversed P buffer; end pad = 0)
kT and v are produced directly in reversed time order by using a reversed
copy of x^T in their matmuls.  y matmuls consume PT/PST (xbar-transposed
P/PS tiles); the softmax denominator Z rides along as a ones-column of v.
k-bias is dropped (softmax shift-invariant), v-bias folded into bproj on
host.  All matmuls run bf16 (full PE rate); logits/exp in fp32; the 1/Z
normalization uses an exact hi/lo bf16 split.
"""

import numpy as np
from contextlib import ExitStack

import concourse.bass as bass
import concourse.tile as tile
from concourse import mybir, bacc
from concourse.bass import ts, ds
from concourse.masks import make_identity

F32 = mybir.dt.float32
BF16 = mybir.dt.bfloat16
EXP = mybir.ActivationFunctionType.Exp
IDENT = mybir.ActivationFunctionType.Identity
ADD = mybir.AluOpType.add
MULT = mybir.AluOpType.mult
SUB = mybir.AluOpType.subtract

NEG_BIG = -1e30


class Cfg:
    def __init__(self, T=1024, H=16, HD=64, CHUNK=512):
        self.T, self.H, self.HD, self.CHUNK = T, H, HD, CHUNK
        self.C = H * HD
        self.NP = H // 2              # head pairs
        self.CT = self.C // 128       # c tiles
        self.TT = T // 128            # t tiles
        self.G = CHUNK // 128         # t-tiles per chunk group
        self.NTC = T // CHUNK         # chunk groups
        assert self.C % 128 == 0 and T % CHUNK == 0 and CHUNK % 128 == 0
        assert HD == 64


def build(nc: bass.Bass, cfg: Cfg, io: dict):
    T, H, HD, C = cfg.T, cfg.H, cfg.HD, cfg.C
    CH = cfg.CHUNK
    NCH = T // CH
    PAD = CH               # qE front pad (causal mask) / P end pad
    QW = PAD + T           # qE dram row length
    PW = T + PAD           # P dram row length
    HD1 = HD + 1

    out_d = io["out"]

    with ExitStack() as big:
        tc = big.enter_context(tile.TileContext(nc))
        const = big.enter_context(tc.tile_pool(name="const", bufs=1))
        persist = big.enter_context(tc.tile_pool(name="persist", bufs=1))
        dscr = big.enter_context(tc.tile_pool(name="dscr", bufs=1, space="DRAM"))

        # ---- constants ----
        embkT2 = const.tile([128, T], BF16)
        nc.sync.dma_start(embkT2[:], io["embkT2"])
        embv = const.tile([128, cfg.TT, HD1], BF16)
        nc.sync.dma_start(embv[:, :, :HD],
                          io["embv_bf"].rearrange("(o p) d -> p o d", p=128))
        erow = embv.ap[0][0]
        embv_zcol = bass.AP(embv.tensor, embv.offset + HD,
                            [[erow, 128], [HD1, cfg.TT]])
        nc.vector.memset(embv_zcol, 0.0)
        bq = const.tile([128, cfg.CT], F32)
        nc.sync.dma_start(bq[:], io["bq"].rearrange("(o p) -> p o", p=128))
        bproj = const.tile([1, C], BF16)
        nc.sync.dma_start(bproj[:], io["bproj_eff"].unsqueeze(0))
        ones = const.tile([1, 128], BF16)
        nc.vector.memset(ones[:], 1.0)
        padneg = const.tile([128, PAD], BF16)
        nc.vector.memset(padneg[:], NEG_BIG)
        padzero = const.tile([128, PAD], BF16)
        nc.vector.memset(padzero[:], 0.0)
        ident = const.tile([128, 128], BF16)
        make_identity(nc, ident[:])

        # ---- persistent activations ----
        qT = persist.tile([128, cfg.NP, T], BF16, tag="qT")
        kTr = persist.tile([128, cfg.NP, T], BF16, tag="kTr")
        v_sb = persist.tile([128, cfg.TT, H * HD1], BF16, tag="v")
        yT = persist.tile([128, cfg.CT, T], BF16, tag="yT")

        # ---- DRAM skew scratch: [parity][head-of-pair][ti] row-blocks ----
        qe_d = dscr.tile([2, 2, cfg.TT, 128, QW], BF16, name="qe_d")
        p_d = dscr.tile([2, 2, cfg.TT, 128, PW], BF16, name="p_d")
        for par in range(2):
            for hh in range(2):
                for ti in range(cfg.TT):
                    nc.sync.dma_start(qe_d[par, hh, ti, :, 0:PAD], padneg[:])
                    nc.sync.dma_start(p_d[par, hh, ti, :, T:T + PAD], padzero[:])

        def dram_diag(dtile, par, hh, ti, col0, n, pstep):
            """AP over a dram scratch row-block: [p, j] -> (p, col0+pstep*p+j)."""
            a = dtile[par, hh, ti]
            rowlen = a.ap[-2][0]
            return bass.AP(a.tensor, a.offset + col0,
                           [[rowlen + pstep, 128], [1, n]])

        # ================= phase 1: projections =================
        with ExitStack() as ph1:
            p1w = ph1.enter_context(tc.tile_pool(name="p1w", bufs=1))
            wpool = ph1.enter_context(tc.tile_pool(name="wls", bufs=3))
            psum1 = ph1.enter_context(
                tc.tile_pool(name="psum1", bufs=4, space="PSUM"))

            xTs = p1w.tile([128, cfg.CT, T], BF16, tag="xT")
            nc.sync.dma_start(xTs[:], io["xT"].rearrange("(o p) t -> p o t", p=128))
            xTr = p1w.tile([128, cfg.CT, T], BF16, tag="xTr")
            nc.sync.dma_start(xTr[:],
                              io["xTrev"].rearrange("(o p) t -> p o t", p=128))

            # qT[j, t] (forward t) / kTr[j, c] (reversed t)
            for which, rhs_sb, dst in (("q", xTs, qT), ("k", xTr, kTr)):
                wd = io["wq"] if which == "q" else io["wk"]
                for jp in range(cfg.CT):
                    wsl = wpool.tile([128, cfg.CT, 128], BF16, tag="wqk")
                    nc.sync.dma_start(
                        wsl[:],
                        wd[:, ts(jp, 128)].rearrange("(o p) j -> p o j", p=128))
                    for tch in range(NCH):
                        ps = psum1.tile([128, CH], F32, tag="ps1")
                        for ct in range(cfg.CT):
                            nc.tensor.matmul(
                                ps[:],
                                lhsT=wsl[:, ct, :],
                                rhs=rhs_sb[:, ct, ts(tch, CH)],
                                start=(ct == 0), stop=(ct == cfg.CT - 1))
                        if which == "q":
                            nc.scalar.activation(
                                dst[:, jp, ts(tch, CH)], ps[:], IDENT,
                                bias=bq[:, jp: jp + 1], scale=1.0)
                        else:
                            nc.scalar.mul(
                                dst[:, jp, ts(tch, CH)], ps[:],
                                float(1.0 / np.sqrt(HD)))

            # v in reversed time order: vr[c, j] = v[T-1-c, j]
            VJ = 256
            nhv = VJ // HD
            for jc in range(C // VJ):
                wsl = wpool.tile([128, cfg.CT, VJ], BF16, tag="wv")
                nc.sync.dma_start(
                    wsl[:],
                    io["wv"][:, ts(jc, VJ)].rearrange("(o p) j -> p o j", p=128))
                for ci in range(cfg.TT):
                    ps = psum1.tile([128, VJ], F32, tag="psv")
                    for ct in range(cfg.CT):
                        nc.tensor.matmul(
                            ps[:],
                            lhsT=xTr[:, ct, ts(ci, 128)],
                            rhs=wsl[:, ct, :],
                            start=(ct == 0), stop=(ct == cfg.CT - 1))
                    vrow = v_sb.ap[0][0]
                    dst = bass.AP(
                        v_sb.tensor,
                        v_sb.offset + ci * (H * HD1) + jc * nhv * HD1,
                        [[vrow, 128], [HD1, nhv], [1, HD]])
                    src = bass.AP(ps.tensor, ps.offset,
                                  [[ps.ap[0][0], 128], [HD, nhv], [1, HD]])
                    nc.scalar.copy(out=dst, in_=src)
            vrow = v_sb.ap[0][0]
            onescol = bass.AP(
                v_sb.tensor, v_sb.offset + HD,
                [[vrow, 128], [H * HD1, cfg.TT], [HD1, H]])
            nc.vector.memset(onescol, 1.0)

        # ================= phase 2: attention =================
        with ExitStack() as ph2:
            st_pool = ph2.enter_context(tc.tile_pool(name="st", bufs=3))
            p_pool = ph2.enter_context(tc.tile_pool(name="pbuf", bufs=2))
            sm_pool = ph2.enter_context(tc.tile_pool(name="sm", bufs=3))
            tr_pool = ph2.enter_context(tc.tile_pool(name="tr", bufs=2))
            zr_pool = ph2.enter_context(tc.tile_pool(name="zr", bufs=2))
            psA = ph2.enter_context(tc.tile_pool(name="psA", bufs=2, space="PSUM"))
            psE = ph2.enter_context(tc.tile_pool(name="psE", bufs=1, space="PSUM"))
            psB = ph2.enter_context(tc.tile_pool(name="psB", bufs=1, space="PSUM"))
            psY = ph2.enter_context(tc.tile_pool(name="psY", bufs=2, space="PSUM"))
            psT = ph2.enter_context(tc.tile_pool(name="psT", bufs=2, space="PSUM"))

            for hp in range(cfg.NP):
                par = hp % 2
                for tcg in range(cfg.NTC):
                    SR = CH * (tcg + 1)            # rect extent
                    nsc = tcg + 1
                    nst = cfg.G * nsc              # 128-blocks in rect
                    PT = {}
                    PST = {}
                    for hh in range(2):
                        PT[hh] = tr_pool.tile([128, cfg.TT, CH], BF16,
                                              tag=f"PT{hh}", name=f"PT{hh}")
                        PST[hh] = tr_pool.tile([128, cfg.TT, CH], BF16,
                                               tag=f"PST{hh}", name=f"PST{hh}")
                    for tloc in range(cfg.G):
                        ti = tcg * cfg.G + tloc
                        E = 128 * (ti + 1)         # causal extent
                        for hh in range(2):
                            lo, hi = 64 * hh, 64 * hh + 64
                            qlhs = qT[lo:hi, hp, ts(ti, 128)]

                            # ---- qE chunks -> dram (forward r coords)
                            qes = st_pool.tile([128, T], BF16, tag="qes")
                            for u in range((E + CH - 1) // CH):
                                w = min(CH, E - CH * u)
                                pse = psE.tile([128, CH], F32, tag="psE")
                                nc.tensor.matmul(
                                    pse[:, :w], lhsT=qlhs,
                                    rhs=embkT2[lo:hi, ds(CH * u, w)],
                                    start=True, stop=True)
                                nc.scalar.copy(out=qes[:, ds(CH * u, w)],
                                               in_=pse[:, :w])
                            nc.scalar.dma_start(
                                qe_d[par, hh, ti, :, ds(PAD, E)], qes[:, :E])

                            # ---- logits/P in reversed-s coords (rect buffer)
                            pb = p_pool.tile([128, T], BF16, tag="pbuf")
                            if SR > E:
                                nc.vector.memset(pb[:, :SR - E], 0.0)
                            a2 = sm_pool.tile([128, T], BF16, tag="a2")
                            nc.sync.dma_start(
                                a2[:, :E],
                                dram_diag(qe_d, par, hh, ti,
                                          PAD + 128 * ti - T + 1 + (T - E),
                                          E, 1))
                            for u in range(nsc):
                                clo = T - SR + CH * u
                                chi = clo + CH
                                wlo = max(clo, T - E)
                                wv = chi - wlo
                                if wv <= 0:
                                    continue
                                psa = psA.tile([128, CH], F32, tag="psA")
                                nc.tensor.matmul(
                                    psa[:, :wv], lhsT=qlhs,
                                    rhs=kTr[lo:hi, hp, ds(wlo, wv)],
                                    start=True, stop=True)
                                af = sm_pool.tile([128, CH], F32, tag="af")
                                nc.vector.tensor_tensor(
                                    af[:, :wv], psa[:, :wv],
                                    a2[:, ds(wlo - (T - E), wv)], ADD)
                                nc.scalar.activation(
                                    pb[:, ds(wlo - (T - SR), wv)],
                                    af[:, :wv], EXP)
                            # valid P region -> dram (reversed c coords)
                            nc.scalar.dma_start(
                                p_d[par, hh, ti, :, ds(T - E, E)],
                                pb[:, ds(SR - E, E)])

                            # ---- PT transposes on PE, batched copy-out
                            for u in range(nsc):
                                pst_ps = psT.tile([128, CH], BF16, tag="psT")
                                for sj in range(cfg.G):
                                    nc.tensor.transpose(
                                        pst_ps[:, ts(sj, 128)],
                                        pb[:, ds(u * CH + sj * 128, 128)],
                                        ident[:])
                                prow = PT[hh].ap[0][0]
                                dst = bass.AP(
                                    PT[hh].tensor,
                                    PT[hh].offset + (u * cfg.G) * CH
                                    + tloc * 128,
                                    [[prow, 128], [CH, cfg.G], [1, 128]])
                                nc.vector.tensor_copy(out=dst, in_=pst_ps[:])

                            # ---- skew#2 (PS, forward r) + PE transposes
                            psk = sm_pool.tile([128, T], BF16, tag="psk")
                            if E < SR:
                                nc.vector.memset(psk[:, E:SR], 0.0)
                            nc.sync.dma_start(
                                psk[:, :E],
                                dram_diag(p_d, par, hh, ti,
                                          T - 1 - 128 * ti, E, -1))
                            for u2 in range(nsc):
                                pst_ps = psT.tile([128, CH], BF16, tag="psT")
                                for rj in range(cfg.G):
                                    nc.tensor.transpose(
                                        pst_ps[:, ts(rj, 128)],
                                        psk[:, ds(u2 * CH + rj * 128, 128)],
                                        ident[:])
                                prow = PST[hh].ap[0][0]
                                dst = bass.AP(
                                    PST[hh].tensor,
                                    PST[hh].offset + (u2 * cfg.G) * CH
                                    + tloc * 128,
                                    [[prow, 128], [CH, cfg.G], [1, 128]])
                                nc.vector.tensor_copy(out=dst, in_=pst_ps[:])

                    # ---- y matmuls for this (pair, chunk group)
                    for hh in range(2):
                        h = 2 * hp + hh
                        psy = psY.tile([65, CH], F32, tag="psY")
                        for si in range(nst):
                            nc.tensor.matmul(
                                psy[:, :],
                                lhsT=v_sb[:, cfg.TT - nst + si,
                                          ds(h * HD1, HD1)],
                                rhs=PT[hh][:, si, :],
                                start=(si == 0), stop=False)
                        for si in range(nst):
                            nc.tensor.matmul(
                                psy[:, :],
                                lhsT=embv[:, si, :],
                                rhs=PST[hh][:, si, :],
                                start=False, stop=(si == nst - 1))
                        # normalize: yT = psy[0:64] / Z, Z = psy[64]
                        rz = zr_pool.tile([1, CH], F32, tag="rz")
                        nc.vector.reciprocal(rz[:], psy[64:65, :])
                        rzh = zr_pool.tile([1, CH], BF16, tag="rzh")
                        nc.vector.tensor_copy(out=rzh[:], in_=rz[:])
                        rzl = zr_pool.tile([1, CH], F32, tag="rzl")
                        nc.vector.tensor_tensor(rzl[:], rz[:], rzh[:], SUB)
                        rzlb = zr_pool.tile([1, CH], BF16, tag="rzlb")
                        nc.vector.tensor_copy(out=rzlb[:], in_=rzl[:])
                        psb = psB.tile([64, CH], F32, tag="psB")
                        nc.tensor.matmul(psb[:], lhsT=ones[:, :HD], rhs=rzh[:],
                                         start=True, stop=False)
                        nc.tensor.matmul(psb[:], lhsT=ones[:, :HD],
                                         rhs=rzlb[:], start=False, stop=True)
                        zb = zr_pool.tile([64, CH], F32, tag="zb")
                        nc.scalar.copy(out=zb[:], in_=psb[:])
                        co, po = divmod(h * HD, 128)
                        nc.vector.tensor_tensor(
                            yT[po:po + HD, co, ts(tcg, CH)],
                            psy[:HD, :], zb[:], MULT)

        # ================= phase 3: output projection =================
        with ExitStack() as ph3:
            p3 = ph3.enter_context(tc.tile_pool(name="p3", bufs=3))
            p3w = ph3.enter_context(tc.tile_pool(name="p3w", bufs=1))
            psum3 = ph3.enter_context(
                tc.tile_pool(name="psum3", bufs=4, space="PSUM"))
            wp_s = p3w.tile([128, cfg.CT, C], BF16, tag="wp")
            nc.sync.dma_start(
                wp_s[:], io["wproj"].rearrange("(o p) j -> p o j", p=128))
            for tt in range(cfg.TT):
                for jc in range(C // CH):
                    ps = psum3.tile([128, CH], F32, tag="ps3")
                    nc.tensor.matmul(
                        ps[:], lhsT=ones[:, :128],
                        rhs=bproj[:, ts(jc, CH)],
                        start=True, stop=False)
                    for ct in range(cfg.CT):
                        nc.tensor.matmul(
                            ps[:],
                            lhsT=yT[:, ct, ts(tt, 128)],
                            rhs=wp_s[:, ct, ts(jc, CH)],
                            start=False, stop=(ct == cfg.CT - 1))
                    ob = p3.tile([128, CH], F32, tag="ob")
                    nc.scalar.copy(out=ob[:], in_=ps[:])
                    nc.sync.dma_start(out_d[ts(tt, 128), ts(jc, CH)], ob[:])


def make_nc(cfg: Cfg, debug=False, compile=True):
    nc = bacc.Bacc("TRN2", target_bir_lowering=False, debug=debug)
    T, C = cfg.T, cfg.C
    io = {}
    io["xT"] = nc.declare_dram_parameter("xT", [C, T], BF16, isOutput=False).ap()
    io["xTrev"] = nc.declare_dram_parameter("xTrev", [C, T], BF16,
                                            isOutput=False).ap()
    io["wq"] = nc.declare_dram_parameter("wq", [C, C], BF16, isOutput=False).ap()
    io["wk"] = nc.declare_dram_parameter("wk", [C, C], BF16, isOutput=False).ap()
    io["wv"] = nc.declare_dram_parameter("wv", [C, C], BF16, isOutput=False).ap()
    io["wproj"] = nc.declare_dram_parameter("wproj", [C, C], BF16,
                                            isOutput=False).ap()
    io["bq"] = nc.declare_dram_parameter("bq", [C], F32, isOutput=False).ap()
    io["bproj_eff"] = nc.declare_dram_parameter("bproj_eff", [C], BF16,
                                                isOutput=False).ap()
    io["embkT2"] = nc.declare_dram_parameter("embkT2", [128, T], BF16,
                                             isOutput=False).ap()
    io["embv_bf"] = nc.declare_dram_parameter("embv_bf", [T, cfg.HD], BF16,
                                              isOutput=False).ap()
    io["out"] = nc.declare_dram_parameter("out", [T, C], F32, isOutput=True).ap()
    build(nc, cfg, io)
    if compile:
        nc.compile()
    return nc


def host_inputs(cfg: Cfg, x_b, Wattn, battn, Wproj, bproj, embk, embv):
    """Per-core input map (numpy) for batch slice x_b [T, C]."""
    import ml_dtypes
    C = cfg.C
    f32 = np.float32
    bf = ml_dtypes.bfloat16
    bv = battn[2 * C:3 * C].astype(np.float64)
    bproj_eff = (bproj.astype(np.float64)
                 + bv @ Wproj.astype(np.float64)).astype(f32)
    nrep = 128 // cfg.HD
    embkT2 = np.concatenate([embk.T] * nrep, axis=0).astype(f32)
    return {
        "xT": np.ascontiguousarray(x_b.T).astype(bf),
        "xTrev": np.ascontiguousarray(x_b[::-1].T).astype(bf),
        "wq": np.ascontiguousarray(Wattn[:, :C]).astype(bf),
        "wk": np.ascontiguousarray(Wattn[:, C:2 * C]).astype(bf),
        "wv": np.ascontiguousarray(Wattn[:, 2 * C:]).astype(bf),
        "wproj": np.ascontiguousarray(Wproj).astype(bf),
        "bq": np.ascontiguousarray(battn[:C], dtype=f32),
        "bproj_eff": bproj_eff.astype(bf),
        "embkT2": np.ascontiguousarray(embkT2).astype(bf),
        "embv_bf": embv.astype(bf),
    }


def ref_numpy(x_b, Wattn, battn, Wproj, bproj, embk, embv, H):
    """fp64 numpy reference for one batch [T, C]."""
    T, C = x_b.shape
    HD = C // H
    x = x_b.astype(np.float64)
    qkv = x @ Wattn.astype(np.float64) + battn.astype(np.float64)
    q, k, v = np.split(qkv, 3, axis=-1)
    q = q.reshape(T, H, HD).transpose(1, 0, 2)
    k = k.reshape(T, H, HD).transpose(1, 0, 2)
    v = v.reshape(T, H, HD).transpose(1, 0, 2)
    idx = np.arange(T)
    rel = np.tril(idx[:, None] - idx[None, :])
    rk = embk.astype(np.float64)[rel]
    rv = embv.astype(np.float64)[rel]
    att = q @ k.transpose(0, 2, 1) / np.sqrt(HD)
    att = att + np.einsum("htd,tsd->hts", q, rk)
    mask = idx[None, :] <= idx[:, None]
    att = np.where(mask[None], att, -np.inf)
    att = att - att.max(-1, keepdims=True)
    p = np.exp(att)
    p = p / p.sum(-1, keepdims=True)
    y = np.einsum("hts,hsd->htd", p, v) + np.einsum("hts,tsd->htd", p, rv)
    y = y.transpose(1, 0, 2).reshape(T, C)
    return (y @ Wproj.astype(np.float64) + bproj.astype(np.float64))


# ======================================================================
# Harness entry point: full inputs in, full outputs out (B sharded over
# 8 NeuronCores, one batch element per core; no collectives needed).
# ======================================================================

_CACHE = {}


def _get_nc():
    if "nc" not in _CACHE:
        cfg = Cfg(T=1024, H=16, HD=64, CHUNK=512)
        _CACHE["cfg"] = cfg
        _CACHE["nc"] = make_nc(cfg, debug=False)
    return _CACHE["nc"], _CACHE["cfg"]


def kernel(x, Wattn, battn, Wproj, bproj, embk, embv):
    from concourse.bass_utils import run_bass_kernel_spmd

    x = np.asarray(x)
    Wattn, battn = np.asarray(Wattn), np.asarray(battn)
    Wproj, bproj = np.asarray(Wproj), np.asarray(bproj)
    embk, embv = np.asarray(embk), np.asarray(embv)
    B = x.shape[0]
    nc, cfg = _get_nc()
    in_maps = [
        host_inputs(cfg, x[b], Wattn, battn, Wproj, bproj, embk, embv)
        for b in range(B)
    ]
    res = run_bass_kernel_spmd(nc, in_maps, list(range(B)))
    out = np.stack([res.results[i]["out"] for i in range(B)])
    return out.astype(np.float32)

